# revision 29
# baseline (speedup 1.0000x reference)
"""Trainium2 Bass kernel for causal multi-head attention with RoPE.

Full module: qkv = x @ w_qkv; RoPE(q, k); causal softmax attention;
out = attn_out @ w_out.  x: [2, 2048, 1024], 16 heads x 64 dim.

Sharding: 8 cores = 2 batches x 4 head-groups (4 heads/core).  Each core
computes its batch's q/k/v for its heads, runs attention, and produces a
partial [2048, 1024] output through its slice of w_out.  Host sums the 4
partials per batch.

Key layout trick: the head-dim axis of q/k (and the cos/sin tables) is
permuted host-side so RoPE's rotate-half pairs sit 16 partitions apart
within a 32-partition quadrant.  The partition swap then becomes a single
DVE stream_shuffle instead of four SBUF-SBUF DMAs.  q.k dot products are
invariant because q and k share the permutation.
"""

import os
import sys

import numpy as np

for _p in ("/opt/trn_rl_repo", "/root/.axon_site/_ro/trn_rl_repo"):
    if os.path.isdir(_p) and _p not in sys.path:
        sys.path.append(_p)

import concourse.bass as bass
import concourse.mybir as mybir
import concourse.tile as tile
from concourse.masks import make_upper_triangular

F32 = mybir.dt.float32
F32R = mybir.dt.float32r
BF16 = mybir.dt.bfloat16

# Problem constants (hardcoded per contest rules)
B = 2
N = 2048
D = 1024
HEADS = 16
DH = 64
N_CORES = 8
HL = HEADS // (N_CORES // B)  # heads per core = 4

# rotate-half pairing permutation: partition p holds rope dim PERM64[p];
# pairs (d, d+32) land 16 partitions apart inside a 32-partition quadrant.
PERM64 = np.array(
    [p for p in range(16)]
    + [p + 16 for p in range(16, 32)]
    + [p - 16 for p in range(32, 48)]
    + [p for p in range(48, 64)]
)
MASK32 = [(i + 16) % 32 for i in range(32)]


def build_attention_nc(
    n=N,
    d=D,
    hl=HL,
    dh=DH,
    load_dt=BF16,
    mm_dt_attn=BF16,
    mm_dt_out=BF16,
    qt=512,
    warm_mms=12,
    lag=5,
    cap_waits=True,
):
    """Build the per-core Bass module.  All cores run the same program (SPMD)."""
    nc = bass.Bass("TRN2", target_bir_lowering=False, debug=False)

    KC = d // 128          # contraction chunks for qkv production
    CT = (hl * dh) // 128  # column tiles for q/k (2 heads per tile)
    NQT = n // qt          # query tiles
    KPQ = qt // 128        # key chunks per query tile
    NSB = n // 128         # seq blocks / key chunks
    OC = (hl * dh) // 128  # w_out contraction chunks from this core
    assert OC == CT
    NOT = d // 512         # output N tiles
    scale = float(dh) ** -0.5

    xT = nc.dram_tensor("xT", [d, n], load_dt, kind="ExternalInput").ap()
    wq = nc.dram_tensor("wq", [d, hl * dh], load_dt, kind="ExternalInput").ap()
    wk = nc.dram_tensor("wk", [d, hl * dh], load_dt, kind="ExternalInput").ap()
    wv = nc.dram_tensor("wv", [d, hl * dh], load_dt, kind="ExternalInput").ap()
    wo = nc.dram_tensor("wo", [hl * dh, d], mm_dt_out, kind="ExternalInput").ap()
    cosT = nc.dram_tensor("cosT", [128, n], BF16, kind="ExternalInput").ap()
    sinT = nc.dram_tensor("sinT", [128, n], BF16, kind="ExternalInput").ap()
    sel = nc.dram_tensor("sel", [hl, hl * dh], F32, kind="ExternalInput").ap()
    outp = nc.dram_tensor("out_partial", [n, d], F32, kind="ExternalOutput").ap()

    with tile.TileContext(nc) as tc:
        with tc.tile_pool(name="pers", bufs=1) as pers:
            # Persistent SBUF tensors alive from production through attention
            qT_sb = [pers.tile([128, n], mm_dt_attn, tag=f"qT{i}", name=f"qT{i}") for i in range(CT)]
            kT_sb = [pers.tile([128, n], mm_dt_attn, tag=f"kT{i}", name=f"kT{i}") for i in range(CT)]
            # v in natural layout, ones column appended per head
            v_sb = [pers.tile([128, hl, dh + 1], mm_dt_attn, tag=f"v{i}", name=f"v{i}") for i in range(NSB)]
            onec_sb = pers.tile([128, 1], F32, tag="onec", name="onec")
            tri_sb = pers.tile([128, 128], F32, tag="tri", name="tri")
            # head-selector rows for the PE-broadcast of softmax denominators:
            # sel4[:, h, :].T @ rs4 broadcasts rs4 row h to 64 partitions.
            sel4 = pers.tile([hl, hl, dh], F32R, tag="sel4", name="sel4")
            self_f = pers.tile([hl, hl, dh], F32, tag="self", name="self")

            nc.vector.memset(onec_sb, 1.0)
            make_upper_triangular(nc, tri_sb[:], val=1.0, diag=True)
            nc.sync.dma_start(self_f, sel.rearrange("p (h e) -> p h e", h=hl))
            nc.vector.tensor_copy(sel4, self_f)

            # e_t pool pre-allocated so its SBUF never aliases the staging
            # x/w tiles (otherwise the first exps wait for all of stage B/C).
            expp = tc.alloc_tile_pool(name="expp", bufs=12)

            # ---- Stages B/C: produce qT, kT (RoPE'd) and v (+ones) ----
            with tc.tile_pool(name="stg", bufs=1) as stg:
                x_sb = [stg.tile([128, n], load_dt, tag=f"x{kc}", name=f"x{kc}") for kc in range(KC)]
                wq_sb = stg.tile([128, KC, hl * dh], load_dt, tag="wq", name="wq")
                wk_sb = stg.tile([128, KC, hl * dh], load_dt, tag="wk", name="wk")
                wv_sb = stg.tile([128, KC, hl * dh], load_dt, tag="wv", name="wv")
                cos_sb = stg.tile([128, n], BF16, tag="cos", name="cos")
                sin_sb = stg.tile([128, n], BF16, tag="sin", name="sin")

                # PE warm-up while the first DMAs land: keeps HAM clocked up.
                warm = stg.tile([128, 512], load_dt, tag="warm", name="warm")
                warmf = stg.tile([128, 512], F32, tag="warmf", name="warmf")
                nc.vector.memset(warmf, 0.0)
                nc.vector.tensor_copy(warm, warmf)
                with tc.tile_pool(name="pswarm", bufs=1, space="PSUM") as pswarm:
                    wps = pswarm.tile([128, 512], F32, tag="wps", name="wps")
                    for i in range(warm_mms):
                        nc.tensor.matmul(
                            wps, warm[:, 0:128], warm,
                            start=(i == 0), stop=(i == warm_mms - 1),
                        )

                # DMA issue order == arrival order (single sync HW queue):
                # weights for q/k first, then x in half-chunks to pace the
                # contraction loop, tables + wv later (needed only at RoPE).
                nc.sync.dma_start(wq_sb, wq.rearrange("(kc p) m -> p kc m", p=128))
                nc.sync.dma_start(wk_sb, wk.rearrange("(kc p) m -> p kc m", p=128))
                hn = n // 2
                for kc in range(KC // 2):
                    nc.sync.dma_start(
                        x_sb[kc][:, 0:hn], xT[kc * 128 : (kc + 1) * 128, 0:hn]
                    )
                    nc.sync.dma_start(
                        x_sb[kc][:, hn:n], xT[kc * 128 : (kc + 1) * 128, hn:n]
                    )
                nc.sync.dma_start(cos_sb, cosT)
                nc.sync.dma_start(sin_sb, sinT)
                for kc in range(KC // 2, KC):
                    nc.sync.dma_start(
                        x_sb[kc][:, 0:hn], xT[kc * 128 : (kc + 1) * 128, 0:hn]
                    )
                    nc.sync.dma_start(
                        x_sb[kc][:, hn:n], xT[kc * 128 : (kc + 1) * 128, hn:n]
                    )
                nc.sync.dma_start(wv_sb, wv.rearrange("(kc p) m -> p kc m", p=128))

                x_mm, wq_mm, wk_mm, wv_mm = x_sb, wq_sb, wk_sb, wv_sb

                with (
                    tc.tile_pool(name="psB", bufs=1, space="PSUM") as psB,
                    tc.tile_pool(name="ropet", bufs=4) as ropet,
                ):
                    def bank(i):
                        return psB.tile([128, qt], F32, tag=f"b{i}", name=f"b{i}")

                    def emit_v_block(sb, bi):
                        psv = bank(bi)
                        for kc in range(KC):
                            nc.tensor.matmul(
                                psv[:, 0 : hl * dh],
                                x_mm[kc][:, sb * 128 : (sb + 1) * 128],
                                wv_mm[:, kc, :],
                                start=(kc == 0),
                                stop=(kc == KC - 1),
                            )
                        nc.scalar.activation(
                            v_sb[sb][:, :, 0:dh],
                            psv[:, 0 : hl * dh].rearrange("p (h e) -> p h e", h=hl),
                            mybir.ActivationFunctionType.Copy,
                        )
                        nc.vector.tensor_copy(
                            v_sb[sb][:, :, dh : dh + 1],
                            onec_sb[:, None, :].to_broadcast([128, hl, 1]),
                        )

                    def rope(ct, qk, st, bk):
                        # psum drained to bf16 by the (idle) scalar engine so
                        # every DVE op below runs in 2-byte fast mode and the
                        # bank frees after a single read.
                        dst = (qT_sb, kT_sb)[qk]
                        sl = slice(st * qt, (st + 1) * qt)
                        raw_t = ropet.tile([128, qt], BF16, tag="raw", name="raw")
                        sh_t = ropet.tile([128, qt], BF16, tag="sh", name="sh")
                        a_t = ropet.tile([128, qt], BF16, tag="a", name="a")
                        nc.scalar.activation(
                            raw_t, bk, mybir.ActivationFunctionType.Copy
                        )
                        nc.vector.stream_shuffle(sh_t, raw_t, MASK32)
                        nc.vector.tensor_tensor(
                            a_t, raw_t, cos_sb[:, sl], mybir.AluOpType.mult
                        )
                        nc.gpsimd.tensor_tensor(
                            sh_t, sh_t, sin_sb[:, sl], mybir.AluOpType.mult
                        )
                        nc.vector.tensor_tensor(
                            dst[ct][:, sl], a_t, sh_t, mybir.AluOpType.add
                        )

                    def qk_group(ct, qk, st, w_mm):
                        bk = bank(qk * 4 + st)
                        csl = slice(ct * 128, (ct + 1) * 128)
                        for kc in range(KC):
                            nc.tensor.matmul(
                                bk,
                                w_mm[:, kc, csl],
                                x_mm[kc][:, st * qt : (st + 1) * qt],
                                start=(kc == 0),
                                stop=(kc == KC - 1),
                            )
                        return bk

                    # ct0: q and k interleaved per contraction chunk so the PE
                    # tracks the x DMA stream; each half-chunk of x feeds four
                    # matmuls.
                    ps_ct0 = {}
                    for st in range(NQT):
                        ps_ct0[(0, st)] = bank(st)
                        ps_ct0[(1, st)] = bank(4 + st)
                    for kc in range(KC):
                        for sts in ((0, 1), (2, 3)):
                            for qk, w_mm in ((0, wq_mm), (1, wk_mm)):
                                for st in sts:
                                    nc.tensor.matmul(
                                        ps_ct0[(qk, st)],
                                        w_mm[:, kc, 0:128],
                                        x_mm[kc][:, st * qt : (st + 1) * qt],
                                        start=(kc == 0),
                                        stop=(kc == KC - 1),
                                    )
                    # RoPE ct0 off PSUM; weave v blocks 0..7 and all of ct1's
                    # qk accumulations into the freed banks to keep the PE fed.
                    ps_ct1 = {}
                    for qk, w_mm in ((0, wq_mm), (1, wk_mm)):
                        for st in range(NQT):
                            rope(0, qk, st, ps_ct0[(qk, st)])
                            emit_v_block(qk * 4 + st, qk * 4 + st)
                            ps_ct1[(qk, st)] = qk_group(1, qk, st, w_mm)
                    # RoPE ct1: k first so v blocks 8..15 cycle through banks
                    # b4..b7 only; the q banks b0..b3 then free early for the
                    # attention-stage PSUM pools.
                    order = [(1, s) for s in range(NQT)] + [(0, s) for s in range(NQT)]
                    for vi, (qk, st) in enumerate(order):
                        rope(1, qk, st, ps_ct1[(qk, st)])
                        emit_v_block(8 + vi, 4 + (vi % 4))

            # ---- Stages D/E/F woven per query tile ----
            pers2 = tc.alloc_tile_pool(name="pers2", bufs=1)
            u_sb = [
                [pers2.tile([dh + 1, qt], F32, tag=f"u{h}_{t}", name=f"u{h}_{t}") for t in range(NQT)]
                for h in range(hl)
            ]
            outT_sb = [pers2.tile([128, n], mm_dt_out, tag=f"oT{i}", name=f"oT{i}") for i in range(CT)]
            wo_sb = pers2.tile([128, OC, d], mm_dt_out, tag="wo", name="wo")
            nc.sync.dma_start(wo_sb, wo.rearrange("(kc p) m -> p kc m", p=128))

            qT_mm, kT_mm, v_mm = qT_sb, kT_sb, v_sb
            oT_mm, wo_mm = outT_sb, wo_sb
            NHP = hl // 2  # head pairs (one q/k column tile each)
            with (
                tc.tile_pool(name="psS", bufs=2, space="PSUM") as psS,
                tc.tile_pool(name="psAV", bufs=1, space="PSUM") as psAV,
                tc.tile_pool(name="bcp", bufs=3) as bcp,
                tc.tile_pool(name="fo", bufs=6) as fo,
            ):
                def make_f_thunk(t, sb):
                    # F for one 128-row block, borrowing a scores-psum
                    # generation so it can weave into the next tile's chunk
                    # loop without its own PSUM banks.
                    def thunk():
                        pss = psS.tile([128, 2, qt], F32, tag="s", name="fps")
                        for nt in range(NOT):
                            for kc in range(OC):
                                nc.tensor.matmul(
                                    pss[:, nt, :],
                                    oT_mm[kc][:, sb * 128 : (sb + 1) * 128],
                                    wo_mm[:, kc, nt * 512 : (nt + 1) * 512],
                                    start=(kc == 0),
                                    stop=(kc == OC - 1),
                                )
                        o_t = fo.tile([128, d], F32, tag="ot", name="ot")
                        nc.vector.tensor_copy(
                            o_t.rearrange("p (a b) -> p a b", a=NOT), pss
                        )
                        nc.sync.dma_start(outp[sb * 128 : (sb + 1) * 128, :], o_t)
                    return thunk

                pending_f = []
                for t in range(NQT):
                    pav = [psAV.tile([dh + 1, qt], F32, tag=f"av{h}", name=f"av{h}") for h in range(hl)]
                    ncc = KPQ * (t + 1)
                    e_ts = {}

                    def emit_scores(c, t=t, e_ts=e_ts):
                        j = c - KPQ * t
                        lo = max(0, j * 128)
                        for hp in range(NHP):
                            pss = psS.tile([128, 2, qt], F32, tag="s", name="s")
                            for g in range(2):
                                bp = 64 * g
                                nc.tensor.matmul(
                                    pss[:, g, lo:qt],
                                    kT_mm[hp][bp : bp + dh, c * 128 : (c + 1) * 128],
                                    qT_mm[hp][bp : bp + dh, t * qt + lo : (t + 1) * qt],
                                    start=True,
                                    stop=True,
                                )
                            e_t = expp.tile([128, 2, qt], mm_dt_attn, tag="e", name="e")
                            nc.scalar.activation(
                                e_t[:, :, lo:qt], pss[:, :, lo:qt],
                                mybir.ActivationFunctionType.Exp, scale=scale,
                            )
                            if j >= 0:
                                nc.gpsimd.tensor_tensor(
                                    e_t[:, :, lo : lo + 128],
                                    e_t[:, :, lo : lo + 128],
                                    tri_sb[:, None, :].to_broadcast([128, 2, 128]),
                                    mybir.AluOpType.mult,
                                )
                            e_ts[(c, hp)] = e_t

                    def emit_av_h(c, hp, g, t=t, e_ts=e_ts, pav=pav, ncc=ncc):
                        lo = max(0, (c - KPQ * t) * 128)
                        e_t = e_ts[(c, hp)]
                        h = 2 * hp + g
                        nc.tensor.matmul(
                            pav[h][:, lo:qt],
                            v_mm[c][:, h, :],
                            e_t[:, g, lo:qt],
                            start=(c == 0),
                            stop=(c == ncc - 1),
                        )
                        if g == 1:
                            e_ts.pop((c, hp))

                    def emit_av(c):
                        for hp in range(NHP):
                            for g in range(2):
                                emit_av_h(c, hp, g)

                    ntrail = min(lag, ncc)
                    for c in range(ncc):
                        if c >= lag:
                            emit_av(c - lag)
                        emit_scores(c)
                        if 3 <= c < 3 + len(pending_f):
                            pending_f[c - 3]()
                    for fth in pending_f[max(0, ncc - 3) :]:
                        fth()
                    pending_f = []
                    # trailing AVs grouped per head so each head's E chain can
                    # start while later heads still accumulate
                    rs4 = bcp.tile([hl, qt], F32, tag="rs4", name="rs4")
                    rsl = bcp.tile([hl, qt], F32, tag="rsl", name="rsl")
                    rs4r = bcp.tile([hl, qt], F32R, tag="rs4r", name="rs4r")
                    for hp in range(NHP):
                        for g in range(2):
                            h = 2 * hp + g
                            for c in range(ncc - ntrail, ncc):
                                emit_av_h(c, hp, g)
                            # ---- E(t, h): copy u, gather rowsum ----
                            nc.vector.tensor_copy(u_sb[h][t], pav[h])
                            nc.sync.dma_start(
                                rs4[h : h + 1, :], u_sb[h][t][dh : dh + 1, :]
                            )
                    if t < NQT - 1:
                        # scalar is exp-saturated mid-stream: invert on DVE
                        with nc.allow_low_precision(reason="f32r recip feeds PE broadcast"):
                            nc.vector.reciprocal(rs4r, rs4)
                    else:
                        # last tile: scalar is idle and Ln/Exp has ~2x less
                        # latency than the DVE reciprocal
                        nc.scalar.activation(
                            rsl, rs4, mybir.ActivationFunctionType.Ln
                        )
                        nc.scalar.activation(
                            rs4r, rsl, mybir.ActivationFunctionType.Exp,
                            scale=-1.0,
                        )
                    # PE broadcast of 1/Z to 64 partitions, then scale on DVE
                    bc_ps = []
                    for h in range(hl):
                        bc = psAV.tile([dh, qt], F32, tag=f"av{h}", name=f"bc{h}")
                        nc.tensor.matmul(bc, sel4[:, h, :], rs4r, start=True, stop=True)
                        bc_ps.append(bc)
                    for sb4 in range(qt // 128):
                        cl = slice(sb4 * 128, (sb4 + 1) * 128)
                        gl = slice(t * qt + sb4 * 128, t * qt + (sb4 + 1) * 128)
                        for h in range(hl):
                            ct_, bp = h // 2, 64 * (h % 2)
                            nc.vector.tensor_tensor(
                                outT_sb[ct_][bp : bp + dh, gl],
                                u_sb[h][t][0:dh, cl],
                                bc_ps[h][:, cl],
                                mybir.AluOpType.mult,
                            )

                    # ---- F(t): deferred into tile t+1's chunk loop ----
                    pending_f = [
                        make_f_thunk(t, sb)
                        for sb in range(t * qt // 128, (t + 1) * qt // 128)
                    ]
                    if t == NQT - 1:
                        for fth in pending_f:
                            fth()
                        pending_f = []
            pers2.release()
            expp.release()
    if cap_waits:
        _cap_matmul_waits(nc)
    return nc


_CAPPED_INSTS = {
    "InstMatmult",
    "InstTensorTensor",
    "InstTensorCopy",
    "InstActivation",
    "InstTensorScalarAffineSelect",
    "InstTensorScalar",
    "InstTensorScalarPtr",
    "InstTensorReduce",
    "InstMemset",
    "InstReciprocal",
    "InstLdweights",
    "InstTensorTensorScan",
    "InstIota",
    "InstDMACopy",
    "InstDrain",
    "InstStreamShuffle",
}


def _cap_matmul_waits(nc, max_keep=1):
    """Walrus codegen allows only one sync-wait per compute instruction
    (S3 struct wait slots).  Move excess waits onto NoOps inserted just
    before, on the same engine; engines execute in order so the semantics
    are identical."""
    nop_id = 0
    for f in nc.m.functions:
        for blk in f.blocks:
            insts = blk.instructions
            idx = 0
            while idx < len(insts):
                inst = insts[idx]
                if (
                    type(inst).__name__ in _CAPPED_INSTS
                    and inst.sync_info is not None
                    and len(inst.sync_info.on_wait or []) > max_keep
                ):
                    waits = list(inst.sync_info.on_wait)
                    extra, keep = waits[:-max_keep], waits[-max_keep:]
                    inst.sync_info = mybir.SyncInfo(
                        on_wait=keep, on_update=list(inst.sync_info.on_update or [])
                    )
                    for w in extra:
                        nop = mybir.InstNoOp(name=f"I-mmwait-nop-{nop_id}")
                        nop_id += 1
                        nop.engine = inst.engine
                        nop.sync_info = mybir.SyncInfo(on_wait=[w], on_update=[])
                        insts.insert(idx, nop)
                        idx += 1
                idx += 1


def _rope_tables(n, dh):
    """Host-side RoPE tables: transposed, 2-head-stacked, permuted by PERM64,
    sign folded into sin."""
    inv_freq = 1.0 / (10000.0 ** (np.arange(0, dh, 2, dtype=np.float32) / dh))
    t = np.arange(n, dtype=np.float32)
    freqs = np.outer(inv_freq, t).astype(np.float32)  # [dh/2, n]
    cos64 = np.cos(np.concatenate([freqs, freqs], axis=0))  # [dh, n]
    sin64 = np.sin(np.concatenate([freqs, freqs], axis=0))
    sign = np.where(PERM64 < dh // 2, -1.0, 1.0).astype(np.float32)[:, None]
    cosP = cos64[PERM64]
    sinP = sin64[PERM64] * sign
    cosT = np.ascontiguousarray(np.tile(cosP, (128 // dh, 1)), dtype=np.float32)
    sinT = np.ascontiguousarray(np.tile(sinP, (128 // dh, 1)), dtype=np.float32)
    return cosT, sinT


def _permute_heads(w, dh):
    """Permute the per-head output columns of [d, heads*dh] by PERM64."""
    d, m = w.shape
    nh = m // dh
    wp = w.reshape(d, nh, dh)[:, :, PERM64]
    return np.ascontiguousarray(wp.reshape(d, m))


_NC_CACHE = {}


def kernel(x, w_qkv, w_out):
    return run(x, w_qkv, w_out)[0]


def run(x, w_qkv, w_out, trace=False, build_kwargs=None):
    import ml_dtypes
    from concourse.bass_utils import run_bass_kernel_spmd

    x = np.asarray(x, dtype=np.float32)
    w_qkv = np.asarray(w_qkv, dtype=np.float32)
    w_out = np.asarray(w_out, dtype=np.float32)

    bk = dict(build_kwargs or {})
    load_dt = bk.get("load_dt", BF16)
    np_load = ml_dtypes.bfloat16 if load_dt == BF16 else np.float32
    out_dt = bk.get("mm_dt_out", BF16)
    np_out = ml_dtypes.bfloat16 if out_dt == BF16 else np.float32

    cosT, sinT = _rope_tables(N, DH)
    sel_np = np.zeros((HL, HL, DH), dtype=np.float32)
    for h in range(HL):
        sel_np[h, h, :] = 1.0
    sel_np = np.ascontiguousarray(sel_np.reshape(HL, HL * DH))
    wq_full = _permute_heads(w_qkv[:, 0:D], DH).astype(np_load)
    wk_full = _permute_heads(w_qkv[:, D : 2 * D], DH).astype(np_load)
    wv_full = np.ascontiguousarray(w_qkv[:, 2 * D :]).astype(np_load)

    in_maps = []
    for core in range(N_CORES):
        b = core // (N_CORES // B)
        g = core % (N_CORES // B)
        cs = slice(g * HL * DH, (g + 1) * HL * DH)
        in_maps.append(
            {
                "xT": np.ascontiguousarray(x[b].T).astype(np_load),
                "wq": np.ascontiguousarray(wq_full[:, cs]),
                "wk": np.ascontiguousarray(wk_full[:, cs]),
                "wv": np.ascontiguousarray(wv_full[:, cs]),
                "wo": np.ascontiguousarray(w_out[cs, :]).astype(np_out),
                "cosT": cosT.astype(ml_dtypes.bfloat16),
                "sinT": sinT.astype(ml_dtypes.bfloat16),
                "sel": sel_np,
            }
        )

    key = repr(sorted((build_kwargs or {}).items()))
    if key not in _NC_CACHE:
        _NC_CACHE[key] = build_attention_nc(**(build_kwargs or {}))
    nc = _NC_CACHE[key]

    res = run_bass_kernel_spmd(
        nc, in_maps, core_ids=list(range(N_CORES)), trace=trace
    )
    out = np.zeros((B, N, D), dtype=np.float32)
    for core in range(N_CORES):
        out[core // (N_CORES // B)] += res.results[core]["out_partial"]
    return out, res


if __name__ == "__main__":
    rng = np.random.default_rng(0)
    x = rng.standard_normal((B, N, D), dtype=np.float32)
    w_qkv = rng.standard_normal((D, 3 * D), dtype=np.float32) * D**-0.5
    w_out = rng.standard_normal((D, D), dtype=np.float32) * D**-0.5
    out = kernel(x, w_qkv, w_out)
    print("out", out.shape, out.dtype, float(np.abs(out).max()))
